# revision 15
# baseline (speedup 1.0000x reference)
"""AttnDecoderRNN single decode step on 8 Trainium2 NeuronCores.

Strategy (tensor-parallel, per sharding hint):
  - Embedding gather: host-side (single row index into the table).
  - Attention (tiny): replicated on all cores.
  - comb_W GEMV: sharded over its 1024 output rows (128 rows/core).
  - GRU gate GEMVs (W_ih, W_hh): sharded over the 1024-dim contraction
    (each core holds a 128-column slice of both weight matrices and its
    128-slice of x / h); partial gate pre-activations are summed with one
    24 KB AllReduce, after which every core finishes the (cheap) gate
    math redundantly and owns the full h_new.
  - out_W GEMV (the memory-bound bulk, 206 MB): sharded over vocab rows,
    6400 padded rows/core.  log_softmax via local (max, sum-exp) stats +
    a 64 B AllGather, merged on-device with the streaming-logsumexp rule.

All device matmuls contract over the partition dim, so every weight is
pre-transposed/tiled on the host into [*, 128(k), ...] layouts that DMA
contiguously per partition.
"""

import numpy as np

try:
    import concourse.bass as bass  # noqa: F401
except ImportError:  # repo not on sys.path (fresh grading dir)
    import sys

    sys.path.insert(0, "/opt/trn_rl_repo")

from concourse import bacc, bass_isa, bass_utils, mybir, tile

F32 = mybir.dt.float32
AF = mybir.ActivationFunctionType
AX = mybir.AxisListType
ALU = mybir.AluOpType

NCORES = 8
H = 1024
E = 1024
L = 100  # encoder length
V = 50257
VC = 6400  # per-core padded vocab rows
VCH = VC // 128  # 50 vocab chunks per core
KCH = H // 128  # 8 contraction chunks over hidden
NEG_BIG = -1.0e30

_CACHE: dict = {}


def _body(tc, io, mode="full"):
    nc = tc.nc
    gall = [list(range(NCORES))]

    with (
        tc.tile_pool(name="singles", bufs=1) as sg,
        tc.tile_pool(name="wstream", bufs=6) as wp,
        tc.tile_pool(name="dram", bufs=1, space="DRAM") as dp,
    ):
        # ---------------- small input loads (issued first) ----------------
        cat1_sb = sg.tile([128, 16], F32)
        nc.sync.dma_start(out=cat1_sb[:], in_=io["cat1"])
        hfull_sb = sg.tile([128, KCH], F32)
        nc.sync.dma_start(out=hfull_sb[:], in_=io["hfull"])
        hown_sb = sg.tile([128, 1], F32)
        nc.sync.dma_start(out=hown_sb[:], in_=io["hown"])
        enc_sb = sg.tile([L, H], F32)
        nc.sync.dma_start(out=enc_sb[:], in_=io["enc"])
        attnw_sb = sg.tile([128, 16, L], F32)
        nc.sync.dma_start(out=attnw_sb[:], in_=io["attnw"].rearrange("a k b -> k a b"))
        attnb_sb = sg.tile([1, L], F32)
        nc.sync.dma_start(out=attnb_sb[:], in_=io["attnb"])
        combw_sb = sg.tile([128, 16, 128], F32)
        nc.sync.dma_start(out=combw_sb[:], in_=io["combw"].rearrange("a k v -> k a v"))
        combb_sb = sg.tile([128, 1], F32)
        nc.sync.dma_start(out=combb_sb[:], in_=io["combb"])
        wih_sb = sg.tile([128, 24, 128], F32)
        nc.sync.dma_start(out=wih_sb[:], in_=io["wih"].rearrange("c k v -> k c v"))
        whh_sb = sg.tile([128, 24, 128], F32)
        nc.sync.dma_start(out=whh_sb[:], in_=io["whh"].rearrange("c k v -> k c v"))
        brz_sb = sg.tile([128, 16], F32)
        nc.sync.dma_start(out=brz_sb[:], in_=io["brz"])
        bin_sb = sg.tile([128, KCH], F32)
        nc.sync.dma_start(out=bin_sb[:], in_=io["bin"])
        bhn_sb = sg.tile([128, KCH], F32)
        nc.sync.dma_start(out=bhn_sb[:], in_=io["bhn"])
        outb_sb = sg.tile([128, VCH], F32)
        nc.sync.dma_start(out=outb_sb[:], in_=io["outb"])

        one_sb = sg.tile([1, 1], F32)
        nc.vector.memset(one_sb[:], 1.0)

        # persistent SBUF results
        aw_sb = sg.tile([1, L], F32)  # attention weights (row)
        awT_sb = sg.tile([L, 1], F32)  # attention weights (column)
        x2att_sb = sg.tile([128, KCH], F32)  # attn_applied chunks
        x_own_sb = sg.tile([128, 1], F32)  # this core's slice of x
        g_sb = sg.tile([128, 48], F32)  # gi partial (0:24) | gh partial (24:48)
        gsum_sb = sg.tile([128, 2, 24], F32)  # allreduced gi | gh
        hnew_sb = sg.tile([128, KCH], F32)  # full new hidden state
        logits_sb = sg.tile([128, VCH], F32)  # local vocab logits
        e_sb = sg.tile([128, VCH], F32)  # exp(logits - m)
        logp_sb = sg.tile([128, VCH], F32)

        # ---------------- phase B: attention (replicated) ----------------
        with tc.tile_pool(name="psA", bufs=2, space="PSUM") as psA:
            aw_ps = psA.tile([1, L], F32)
            for a in range(16):
                nc.tensor.matmul(
                    aw_ps[:],
                    cat1_sb[:, a : a + 1],
                    attnw_sb[:, a, :],
                    start=(a == 0),
                    stop=(a == 15),
                )
            t_att = sg.tile([1, L], F32)
            nc.vector.tensor_add(t_att[:], aw_ps[:], attnb_sb[:])
            nm_att = sg.tile([1, 1], F32)
            nc.vector.reduce_max(nm_att[:], t_att[:], axis=AX.X, negate=True)
            s_att = sg.tile([1, 1], F32)
            e_att = sg.tile([1, L], F32)
            nc.scalar.activation(
                out=e_att[:], in_=t_att[:], func=AF.Exp,
                bias=nm_att[:], scale=1.0, accum_out=s_att[:],
            )
            r_att = sg.tile([1, 1], F32)
            nc.vector.reciprocal(r_att[:], s_att[:])
            nc.vector.tensor_scalar_mul(aw_sb[:], e_att[:], r_att[:])
            nc.sync.dma_start(out=io["out_aw"], in_=aw_sb[:])

            # transpose attention weights into a column via PE
            awT_ps = psA.tile([L, 1], F32)
            nc.tensor.matmul(awT_ps[:], aw_sb[:], one_sb[:], start=True, stop=True)
            nc.vector.tensor_copy(awT_sb[:], awT_ps[:])

            # attn_applied = enc.T @ attn_weights, chunked over hidden
            ap_ps = psA.tile([128, KCH], F32)
            for m in range(KCH):
                nc.tensor.matmul(
                    ap_ps[:, m : m + 1],
                    enc_sb[:, m * 128 : (m + 1) * 128],
                    awT_sb[:],
                    start=True,
                    stop=True,
                )
            nc.vector.tensor_copy(x2att_sb[:], ap_ps[:])

            # ------------- phase C: comb GEMV (output-sharded) -------------
            xc_ps = psA.tile([128, 1], F32)
            for a in range(16):
                rhs = cat1_sb[:, a : a + 1] if a < 8 else x2att_sb[:, a - 8 : a - 7]
                nc.tensor.matmul(
                    xc_ps[:], combw_sb[:, a, :], rhs, start=(a == 0), stop=(a == 15)
                )
            nc.scalar.activation(
                out=x_own_sb[:], in_=xc_ps[:], func=AF.Relu,
                bias=combb_sb[:], scale=1.0,
            )

        if mode == "attn":
            nc.sync.dma_start(out=io["out_h"], in_=x2att_sb[:])
            nc.sync.dma_start(out=io["out_logp"], in_=outb_sb[:])
            return

        # ------------- phase D: GRU gate partials (contraction-sharded) ----
        with tc.tile_pool(name="psG", bufs=3, space="PSUM") as psG:
            for half, rhs in ((0, x_own_sb), (1, hown_sb)):
                for j in range(3):
                    gp = psG.tile([128, KCH], F32)
                    for a in range(KCH):
                        c = j * KCH + a
                        w = wih_sb if half == 0 else whh_sb
                        nc.tensor.matmul(
                            gp[:, a : a + 1], w[:, c, :], rhs[:],
                            start=True, stop=True,
                        )
                    lo = half * 24 + j * KCH
                    nc.vector.tensor_copy(g_sb[:, lo : lo + KCH], gp[:])

        ar_in = dp.tile([2, 128, 24], F32)
        ar_out = dp.tile([2, 128, 24], F32)
        nc.sync.dma_start(out=ar_in[0], in_=g_sb[:, 0:24])
        nc.sync.dma_start(out=ar_in[1], in_=g_sb[:, 24:48])
        nc.gpsimd.collective_compute(
            "AllReduce", ALU.add, replica_groups=gall,
            ins=[ar_in[:].opt()], outs=[ar_out[:].opt()],
        )
        nc.sync.dma_start(out=gsum_sb[:], in_=ar_out[:].rearrange("g p c -> p g c"))
        gi = gsum_sb[:, 0, :]
        gh = gsum_sb[:, 1, :]

        # ---------------- phase E: gate math (replicated) ----------------
        pre = sg.tile([128, 16], F32)
        nc.vector.tensor_add(pre[:], gi[:, 0:16], gh[:, 0:16])
        nc.vector.tensor_add(pre[:], pre[:], brz_sb[:])
        r_g = sg.tile([128, KCH], F32)
        z_g = sg.tile([128, KCH], F32)
        nc.scalar.activation(out=r_g[:], in_=pre[:, 0:KCH], func=AF.Sigmoid)
        nc.scalar.activation(out=z_g[:], in_=pre[:, KCH:16], func=AF.Sigmoid)
        hn_p = sg.tile([128, KCH], F32)
        nc.vector.tensor_add(hn_p[:], gh[:, 16:24], bhn_sb[:])
        n_p = sg.tile([128, KCH], F32)
        nc.vector.tensor_add(n_p[:], gi[:, 16:24], bin_sb[:])
        nc.vector.tensor_mul(hn_p[:], r_g[:], hn_p[:])
        nc.vector.tensor_add(n_p[:], n_p[:], hn_p[:])
        n_g = sg.tile([128, KCH], F32)
        nc.scalar.activation(out=n_g[:], in_=n_p[:], func=AF.Tanh)
        d_g = sg.tile([128, KCH], F32)
        nc.vector.tensor_sub(d_g[:], hfull_sb[:], n_g[:])
        nc.vector.tensor_mul(d_g[:], z_g[:], d_g[:])
        nc.vector.tensor_add(hnew_sb[:], n_g[:], d_g[:])
        nc.sync.dma_start(out=io["out_h"], in_=hnew_sb[:])

        if mode == "gru":
            nc.sync.dma_start(out=io["out_logp"], in_=outb_sb[:])
            return

        # ------------- phase F: out_W GEMV (vocab-sharded stream) ----------
        with tc.tile_pool(name="psF", bufs=4, space="PSUM") as psF:
            for c in range(VCH):
                wt = wp.tile([128, H], F32)
                nc.sync.dma_start(out=wt[:], in_=io["outw"][c])
                lp = psF.tile([128, 1], F32)
                for a in range(KCH):
                    nc.tensor.matmul(
                        lp[:],
                        wt[:, a * 128 : (a + 1) * 128],
                        hnew_sb[:, a : a + 1],
                        start=(a == 0),
                        stop=(a == KCH - 1),
                    )
                nc.vector.tensor_add(
                    logits_sb[:, c : c + 1], lp[:], outb_sb[:, c : c + 1]
                )

        if mode == "gemv":
            nc.sync.dma_start(out=io["out_logp"], in_=logits_sb[:])
            return

        # ---------------- local log-sum-exp stats ----------------
        mloc = sg.tile([128, 1], F32)
        nc.vector.reduce_max(mloc[:], logits_sb[:], axis=AX.X)
        gmax = sg.tile([128, 1], F32)
        nc.gpsimd.partition_all_reduce(
            gmax[:], mloc[:], channels=128, reduce_op=bass_isa.ReduceOp.max
        )
        nmax = sg.tile([128, 1], F32)
        nc.vector.tensor_scalar_mul(nmax[:], gmax[:], -1.0)
        srow = sg.tile([128, 1], F32)
        nc.scalar.activation(
            out=e_sb[:], in_=logits_sb[:], func=AF.Exp,
            bias=nmax[:], scale=1.0, accum_out=srow[:],
        )
        sall = sg.tile([128, 1], F32)
        nc.gpsimd.partition_all_reduce(
            sall[:], srow[:], channels=128, reduce_op=bass_isa.ReduceOp.add
        )

        if mode == "lse":
            nc.sync.dma_start(out=io["out_logp"], in_=logits_sb[:])
            return

        # ------- cross-core merge: masked AllReduce-add of (m, s) table ----
        # (an AllGather would be natural, but mixing collective kinds in one
        # NEFF wedges the device; a one-hot-masked AllReduce is equivalent)
        oh_sb = sg.tile([1, NCORES, 1], F32)
        nc.sync.dma_start(out=oh_sb[:], in_=io["onehot"])
        ms16 = sg.tile([1, NCORES, 2], F32)
        nc.vector.tensor_scalar_mul(ms16[:, :, 0:1], oh_sb[:], gmax[0:1, :])
        nc.vector.tensor_scalar_mul(ms16[:, :, 1:2], oh_sb[:], sall[0:1, :])
        ms_in = dp.tile([1, NCORES, 2], F32)
        ms_out = dp.tile([1, NCORES, 2], F32)
        nc.sync.dma_start(out=ms_in[:], in_=ms16[:])
        nc.gpsimd.collective_compute(
            "AllReduce", ALU.add, replica_groups=gall,
            ins=[ms_in[:].opt()], outs=[ms_out[:].opt()],
        )
        ms_sb = sg.tile([1, NCORES, 2], F32)
        nc.sync.dma_start(out=ms_sb[:], in_=ms_out[:])

        if mode == "ar2":
            nc.sync.dma_start(
                out=io["out_aw"][:, 0:16].rearrange("p (a b) -> p a b", b=2),
                in_=ms_sb[:],
            )
            nc.sync.dma_start(out=io["out_logp"], in_=logits_sb[:])
            return
        gm = sg.tile([1, 1], F32)
        nc.vector.reduce_max(gm[:], ms_sb[:, :, 0:1], axis=AX.XY)
        ngm = sg.tile([1, 1], F32)
        nc.vector.tensor_scalar_mul(ngm[:], gm[:], -1.0)
        corr = sg.tile([1, NCORES, 1], F32)
        nc.scalar.activation(
            out=corr[:], in_=ms_sb[:, :, 0:1], func=AF.Exp, bias=ngm[:], scale=1.0
        )
        prod = sg.tile([1, NCORES, 1], F32)
        ssum = sg.tile([1, 1], F32)
        # (tensor_tensor_reduce mis-executes on HW via this path; keep it simple)
        nc.vector.tensor_mul(prod[:], corr[:], ms_sb[:, :, 1:2])
        nc.vector.reduce_sum(ssum[:], prod[:], axis=AX.XY)
        lse = sg.tile([1, 1], F32)
        nc.scalar.activation(out=lse[:], in_=ssum[:], func=AF.Ln)
        nc.vector.tensor_add(lse[:], lse[:], gm[:])
        if mode == "merge":
            nc.sync.dma_start(out=io["out_aw"][:, 0:1], in_=lse[:])
            nc.sync.dma_start(out=io["out_logp"], in_=logits_sb[:])
            return

        lse_b = sg.tile([128, 1], F32)
        nc.gpsimd.partition_broadcast(lse_b[:], lse[:])

        nc.vector.tensor_scalar_sub(logp_sb[:], logits_sb[:], lse_b[:])
        nc.sync.dma_start(out=io["out_logp"], in_=logp_sb[:])


def build_module(mode="full"):
    if ("nc", mode) in _CACHE:
        return _CACHE[("nc", mode)]
    nc = bacc.Bacc(
        "TRN2",
        target_bir_lowering=False,
        debug=False,
        enable_asserts=False,
        num_devices=NCORES,
    )

    def din(name, shape):
        return nc.dram_tensor(name, shape, F32, kind="ExternalInput").ap()

    def dout(name, shape):
        return nc.dram_tensor(name, shape, F32, kind="ExternalOutput").ap()

    io = {
        "cat1": din("cat1", [128, 16]),
        "hfull": din("hfull", [128, KCH]),
        "hown": din("hown", [128, 1]),
        "enc": din("enc", [L, H]),
        "attnw": din("attnw", [16, 128, L]),
        "attnb": din("attnb", [1, L]),
        "combw": din("combw", [16, 128, 128]),
        "combb": din("combb", [128, 1]),
        "wih": din("wih", [24, 128, 128]),
        "whh": din("whh", [24, 128, 128]),
        "brz": din("brz", [128, 16]),
        "bin": din("bin", [128, KCH]),
        "bhn": din("bhn", [128, KCH]),
        "outw": din("outw", [VCH, 128, H]),
        "outb": din("outb", [128, VCH]),
        "onehot": din("onehot", [1, NCORES, 1]),
        "out_logp": dout("out_logp", [128, VCH]),
        "out_h": dout("out_h", [128, KCH]),
        "out_aw": dout("out_aw", [1, L]),
    }

    with tile.TileContext(nc) as tc:
        _body(tc, io, mode)
    nc.compile()
    _CACHE[("nc", mode)] = nc
    return nc


def prep_inputs(
    input_token, hidden, encoder_outputs, emb, attn_W, attn_b,
    comb_W, comb_b, W_ih, W_hh, b_ih, b_hh, out_W, out_b,
):
    """Host-side sharding/layout. Returns per-core input maps."""
    f = np.float32
    tok = int(np.asarray(input_token).reshape(-1)[0])
    emb_row = np.asarray(emb[tok], dtype=f).reshape(E)
    hvec = np.asarray(hidden, dtype=f).reshape(H)
    cat1 = np.concatenate([emb_row, hvec]).reshape(16, 128).T.copy()
    hfull = hvec.reshape(KCH, 128).T.copy()
    enc = np.ascontiguousarray(np.asarray(encoder_outputs, dtype=f))
    attnw = np.ascontiguousarray(
        np.asarray(attn_W, dtype=f).T.reshape(16, 128, L)
    )
    attnb = np.asarray(attn_b, dtype=f).reshape(1, L).copy()
    b_ih = np.asarray(b_ih, dtype=f)
    b_hh = np.asarray(b_hh, dtype=f)
    brz = np.concatenate(
        [
            (b_ih[0:H] + b_hh[0:H]).reshape(KCH, 128).T,
            (b_ih[H : 2 * H] + b_hh[H : 2 * H]).reshape(KCH, 128).T,
        ],
        axis=1,
    ).copy()
    bin_ = b_ih[2 * H : 3 * H].reshape(KCH, 128).T.copy()
    bhn = b_hh[2 * H : 3 * H].reshape(KCH, 128).T.copy()

    comb_W = np.asarray(comb_W, dtype=f)
    comb_b = np.asarray(comb_b, dtype=f)
    W_ih = np.asarray(W_ih, dtype=f)
    W_hh = np.asarray(W_hh, dtype=f)

    out_W = np.asarray(out_W, dtype=f)
    out_b = np.asarray(out_b, dtype=f)
    vpad = NCORES * VC
    outw_pad = np.zeros((vpad, H), dtype=f)
    outw_pad[:V] = out_W
    outb_pad = np.full((vpad,), NEG_BIG, dtype=f)
    outb_pad[:V] = out_b

    maps = []
    for i in range(NCORES):
        sl = slice(i * 128, (i + 1) * 128)
        combw_t = np.ascontiguousarray(
            comb_W[sl, :].T.reshape(16, 128, 128)
        )
        wih_t = np.ascontiguousarray(
            W_ih[:, sl].reshape(24, 128, 128).transpose(0, 2, 1)
        )
        whh_t = np.ascontiguousarray(
            W_hh[:, sl].reshape(24, 128, 128).transpose(0, 2, 1)
        )
        shard = out_W_shard = outw_pad[i * VC : (i + 1) * VC]
        outw_t = np.ascontiguousarray(
            shard.reshape(VCH, 128, KCH, 128).transpose(0, 3, 2, 1)
        ).reshape(VCH, 128, H)
        outb_t = outb_pad[i * VC : (i + 1) * VC].reshape(VCH, 128).T.copy()
        maps.append(
            {
                "cat1": cat1,
                "hfull": hfull,
                "hown": hvec[sl].reshape(128, 1).copy(),
                "enc": enc,
                "attnw": attnw,
                "attnb": attnb,
                "combw": combw_t,
                "combb": comb_b[sl].reshape(128, 1).copy(),
                "wih": wih_t,
                "whh": whh_t,
                "brz": brz,
                "bin": bin_,
                "bhn": bhn,
                "outw": outw_t,
                "outb": outb_t,
                "onehot": np.eye(NCORES, dtype=f)[i].reshape(1, NCORES, 1).copy(),
            }
        )
    return maps


def finish_outputs(results):
    """Gather/unshard per-core results into reference-shaped outputs."""
    logp = np.concatenate(
        [np.asarray(results[i]["out_logp"]).T.reshape(VC) for i in range(NCORES)]
    )[:V].reshape(1, V)
    h_new = np.asarray(results[0]["out_h"]).T.reshape(1, 1, H)
    attn_w = np.asarray(results[0]["out_aw"]).reshape(1, L)
    return logp.astype(np.float32), h_new.astype(np.float32), attn_w.astype(np.float32)


def run_on_hw(in_maps, trace=False, mode="full", **kw):
    nc = build_module(mode)
    from concourse.bass_interp import get_hw_module

    old = nc.m
    nc.m = get_hw_module(nc.m)
    try:
        return bass_utils.run_bass_kernel_spmd(
            nc, in_maps, core_ids=list(range(NCORES)), trace=trace, **kw
        )
    finally:
        nc.m = old


def kernel(**inputs):
    in_maps = prep_inputs(**inputs)
    res = run_on_hw(in_maps)
    return finish_outputs(res.results)


# revision 16
# speedup vs baseline: 8.8113x; 8.8113x over previous
"""AttnDecoderRNN single decode step on 8 Trainium2 NeuronCores.

Strategy (tensor-parallel, per sharding hint):
  - Embedding gather: host-side (single row index into the table).
  - Attention (tiny): replicated on all cores.
  - comb_W GEMV: sharded over its 1024 output rows (128 rows/core).
  - GRU gate GEMVs (W_ih, W_hh): sharded over the 1024-dim contraction
    (each core holds a 128-column slice of both weight matrices and its
    128-slice of x / h); partial gate pre-activations are summed with one
    24 KB AllReduce, after which every core finishes the (cheap) gate
    math redundantly and owns the full h_new.
  - out_W GEMV (the memory-bound bulk, 206 MB): sharded over vocab rows,
    6400 padded rows/core.  log_softmax via local (max, sum-exp) stats +
    a 64 B AllGather, merged on-device with the streaming-logsumexp rule.

All device matmuls contract over the partition dim, so every weight is
pre-transposed/tiled on the host into [*, 128(k), ...] layouts that DMA
contiguously per partition.
"""

import numpy as np

try:
    import concourse.bass as bass  # noqa: F401
except ImportError:  # repo not on sys.path (fresh grading dir)
    import sys

    sys.path.insert(0, "/opt/trn_rl_repo")

from concourse import bacc, bass_isa, bass_utils, mybir, tile

F32 = mybir.dt.float32
AF = mybir.ActivationFunctionType
AX = mybir.AxisListType
ALU = mybir.AluOpType

NCORES = 8
H = 1024
E = 1024
L = 100  # encoder length
V = 50257
VC = 6400  # per-core padded vocab rows
VCH = VC // 128  # 50 vocab chunks per core
KCH = H // 128  # 8 contraction chunks over hidden
NEG_BIG = -1.0e30

_CACHE: dict = {}


def _body(tc, io, mode="full", sfx=""):
    nc = tc.nc
    gall = [list(range(NCORES))]

    with (
        tc.tile_pool(name="singles" + sfx, bufs=1) as sg,
        tc.tile_pool(name="wstream" + sfx, bufs=6) as wp,
        tc.tile_pool(name="dram" + sfx, bufs=1, space="DRAM") as dp,
    ):
        # ---------------- small input loads (issued first) ----------------
        cat1_sb = sg.tile([128, 16], F32)
        nc.sync.dma_start(out=cat1_sb[:], in_=io["cat1"])
        hfull_sb = sg.tile([128, KCH], F32)
        nc.sync.dma_start(out=hfull_sb[:], in_=io["hfull"])
        hown_sb = sg.tile([128, 1], F32)
        nc.sync.dma_start(out=hown_sb[:], in_=io["hown"])
        enc_sb = sg.tile([L, H], F32)
        nc.sync.dma_start(out=enc_sb[:], in_=io["enc"])
        attnw_sb = sg.tile([128, 16, L], F32)
        nc.sync.dma_start(out=attnw_sb[:], in_=io["attnw"].rearrange("a k b -> k a b"))
        attnb_sb = sg.tile([1, L], F32)
        nc.sync.dma_start(out=attnb_sb[:], in_=io["attnb"])
        combw_sb = sg.tile([128, 16, 128], F32)
        nc.sync.dma_start(out=combw_sb[:], in_=io["combw"].rearrange("a k v -> k a v"))
        combb_sb = sg.tile([128, 1], F32)
        nc.sync.dma_start(out=combb_sb[:], in_=io["combb"])
        wih_sb = sg.tile([128, 24, 128], F32)
        nc.sync.dma_start(out=wih_sb[:], in_=io["wih"].rearrange("c k v -> k c v"))
        whh_sb = sg.tile([128, 24, 128], F32)
        nc.sync.dma_start(out=whh_sb[:], in_=io["whh"].rearrange("c k v -> k c v"))
        brz_sb = sg.tile([128, 16], F32)
        nc.sync.dma_start(out=brz_sb[:], in_=io["brz"])
        bin_sb = sg.tile([128, KCH], F32)
        nc.sync.dma_start(out=bin_sb[:], in_=io["bin"])
        bhn_sb = sg.tile([128, KCH], F32)
        nc.sync.dma_start(out=bhn_sb[:], in_=io["bhn"])
        outb_sb = sg.tile([128, VCH], F32)
        nc.sync.dma_start(out=outb_sb[:], in_=io["outb"])

        one_sb = sg.tile([1, 1], F32)
        nc.vector.memset(one_sb[:], 1.0)

        # persistent SBUF results
        aw_sb = sg.tile([1, L], F32)  # attention weights (row)
        awT_sb = sg.tile([L, 1], F32)  # attention weights (column)
        x2att_sb = sg.tile([128, KCH], F32)  # attn_applied chunks
        x_own_sb = sg.tile([128, 1], F32)  # this core's slice of x
        g_sb = sg.tile([128, 48], F32)  # gi partial (0:24) | gh partial (24:48)
        gsum_sb = sg.tile([128, 2, 24], F32)  # allreduced gi | gh
        hnew_sb = sg.tile([128, KCH], F32)  # full new hidden state
        logits_sb = sg.tile([128, VCH], F32)  # local vocab logits
        e_sb = sg.tile([128, VCH], F32)  # exp(logits - m)
        logp_sb = sg.tile([128, VCH], F32)

        # ---------------- phase B: attention (replicated) ----------------
        with tc.tile_pool(name="psA" + sfx, bufs=2, space="PSUM") as psA:
            aw_ps = psA.tile([1, L], F32)
            for a in range(16):
                nc.tensor.matmul(
                    aw_ps[:],
                    cat1_sb[:, a : a + 1],
                    attnw_sb[:, a, :],
                    start=(a == 0),
                    stop=(a == 15),
                )
            t_att = sg.tile([1, L], F32)
            nc.vector.tensor_add(t_att[:], aw_ps[:], attnb_sb[:])
            nm_att = sg.tile([1, 1], F32)
            nc.vector.reduce_max(nm_att[:], t_att[:], axis=AX.X, negate=True)
            s_att = sg.tile([1, 1], F32)
            e_att = sg.tile([1, L], F32)
            nc.scalar.activation(
                out=e_att[:], in_=t_att[:], func=AF.Exp,
                bias=nm_att[:], scale=1.0, accum_out=s_att[:],
            )
            r_att = sg.tile([1, 1], F32)
            nc.vector.reciprocal(r_att[:], s_att[:])
            nc.vector.tensor_scalar_mul(aw_sb[:], e_att[:], r_att[:])
            nc.sync.dma_start(out=io["out_aw"], in_=aw_sb[:])

            # transpose attention weights into a column via PE
            awT_ps = psA.tile([L, 1], F32)
            nc.tensor.matmul(awT_ps[:], aw_sb[:], one_sb[:], start=True, stop=True)
            nc.vector.tensor_copy(awT_sb[:], awT_ps[:])

            # attn_applied = enc.T @ attn_weights, chunked over hidden
            ap_ps = psA.tile([128, KCH], F32)
            for m in range(KCH):
                nc.tensor.matmul(
                    ap_ps[:, m : m + 1],
                    enc_sb[:, m * 128 : (m + 1) * 128],
                    awT_sb[:],
                    start=True,
                    stop=True,
                )
            nc.vector.tensor_copy(x2att_sb[:], ap_ps[:])

            # ------------- phase C: comb GEMV (output-sharded) -------------
            xc_ps = psA.tile([128, 1], F32)
            for a in range(16):
                rhs = cat1_sb[:, a : a + 1] if a < 8 else x2att_sb[:, a - 8 : a - 7]
                nc.tensor.matmul(
                    xc_ps[:], combw_sb[:, a, :], rhs, start=(a == 0), stop=(a == 15)
                )
            nc.scalar.activation(
                out=x_own_sb[:], in_=xc_ps[:], func=AF.Relu,
                bias=combb_sb[:], scale=1.0,
            )

        if mode == "attn":
            nc.sync.dma_start(out=io["out_h"], in_=x2att_sb[:])
            nc.sync.dma_start(out=io["out_logp"], in_=outb_sb[:])
            return

        # ------------- phase D: GRU gate partials (contraction-sharded) ----
        with tc.tile_pool(name="psG" + sfx, bufs=3, space="PSUM") as psG:
            for half, rhs in ((0, x_own_sb), (1, hown_sb)):
                for j in range(3):
                    gp = psG.tile([128, KCH], F32)
                    for a in range(KCH):
                        c = j * KCH + a
                        w = wih_sb if half == 0 else whh_sb
                        nc.tensor.matmul(
                            gp[:, a : a + 1], w[:, c, :], rhs[:],
                            start=True, stop=True,
                        )
                    lo = half * 24 + j * KCH
                    nc.vector.tensor_copy(g_sb[:, lo : lo + KCH], gp[:])

        ar_in = dp.tile([2, 128, 24], F32)
        ar_out = dp.tile([2, 128, 24], F32)
        nc.sync.dma_start(out=ar_in[0], in_=g_sb[:, 0:24])
        nc.sync.dma_start(out=ar_in[1], in_=g_sb[:, 24:48])
        nc.gpsimd.collective_compute(
            "AllReduce", ALU.add, replica_groups=gall,
            ins=[ar_in[:].opt()], outs=[ar_out[:].opt()],
        )
        nc.sync.dma_start(out=gsum_sb[:], in_=ar_out[:].rearrange("g p c -> p g c"))
        gi = gsum_sb[:, 0, :]
        gh = gsum_sb[:, 1, :]

        # ---------------- phase E: gate math (replicated) ----------------
        pre = sg.tile([128, 16], F32)
        nc.vector.tensor_add(pre[:], gi[:, 0:16], gh[:, 0:16])
        nc.vector.tensor_add(pre[:], pre[:], brz_sb[:])
        r_g = sg.tile([128, KCH], F32)
        z_g = sg.tile([128, KCH], F32)
        nc.scalar.activation(out=r_g[:], in_=pre[:, 0:KCH], func=AF.Sigmoid)
        nc.scalar.activation(out=z_g[:], in_=pre[:, KCH:16], func=AF.Sigmoid)
        hn_p = sg.tile([128, KCH], F32)
        nc.vector.tensor_add(hn_p[:], gh[:, 16:24], bhn_sb[:])
        n_p = sg.tile([128, KCH], F32)
        nc.vector.tensor_add(n_p[:], gi[:, 16:24], bin_sb[:])
        nc.vector.tensor_mul(hn_p[:], r_g[:], hn_p[:])
        nc.vector.tensor_add(n_p[:], n_p[:], hn_p[:])
        n_g = sg.tile([128, KCH], F32)
        nc.scalar.activation(out=n_g[:], in_=n_p[:], func=AF.Tanh)
        d_g = sg.tile([128, KCH], F32)
        nc.vector.tensor_sub(d_g[:], hfull_sb[:], n_g[:])
        nc.vector.tensor_mul(d_g[:], z_g[:], d_g[:])
        nc.vector.tensor_add(hnew_sb[:], n_g[:], d_g[:])
        nc.sync.dma_start(out=io["out_h"], in_=hnew_sb[:])

        if mode == "gru":
            nc.sync.dma_start(out=io["out_logp"], in_=outb_sb[:])
            return

        # ------------- phase F: out_W GEMV (vocab-sharded stream) ----------
        with tc.tile_pool(name="psF" + sfx, bufs=4, space="PSUM") as psF:
            for c in range(VCH):
                wt = wp.tile([128, H], F32)
                nc.sync.dma_start(out=wt[:], in_=io["outw"][c])
                lp = psF.tile([128, 1], F32)
                for a in range(KCH):
                    nc.tensor.matmul(
                        lp[:],
                        wt[:, a * 128 : (a + 1) * 128],
                        hnew_sb[:, a : a + 1],
                        start=(a == 0),
                        stop=(a == KCH - 1),
                    )
                nc.vector.tensor_add(
                    logits_sb[:, c : c + 1], lp[:], outb_sb[:, c : c + 1]
                )

        if mode == "gemv":
            nc.sync.dma_start(out=io["out_logp"], in_=logits_sb[:])
            return

        # ---------------- local log-sum-exp stats ----------------
        mloc = sg.tile([128, 1], F32)
        nc.vector.reduce_max(mloc[:], logits_sb[:], axis=AX.X)
        gmax = sg.tile([128, 1], F32)
        nc.gpsimd.partition_all_reduce(
            gmax[:], mloc[:], channels=128, reduce_op=bass_isa.ReduceOp.max
        )
        nmax = sg.tile([128, 1], F32)
        nc.vector.tensor_scalar_mul(nmax[:], gmax[:], -1.0)
        srow = sg.tile([128, 1], F32)
        nc.scalar.activation(
            out=e_sb[:], in_=logits_sb[:], func=AF.Exp,
            bias=nmax[:], scale=1.0, accum_out=srow[:],
        )
        sall = sg.tile([128, 1], F32)
        nc.gpsimd.partition_all_reduce(
            sall[:], srow[:], channels=128, reduce_op=bass_isa.ReduceOp.add
        )

        if mode == "lse":
            nc.sync.dma_start(out=io["out_logp"], in_=logits_sb[:])
            return

        # ------- cross-core merge: masked AllReduce-add of (m, s) table ----
        # (an AllGather would be natural, but mixing collective kinds in one
        # NEFF wedges the device; a one-hot-masked AllReduce is equivalent)
        oh_sb = sg.tile([1, NCORES, 1], F32)
        nc.sync.dma_start(out=oh_sb[:], in_=io["onehot"])
        ms16 = sg.tile([1, NCORES, 2], F32)
        nc.vector.tensor_scalar_mul(ms16[:, :, 0:1], oh_sb[:], gmax[0:1, :])
        nc.vector.tensor_scalar_mul(ms16[:, :, 1:2], oh_sb[:], sall[0:1, :])
        ms_in = dp.tile([1, NCORES, 2], F32)
        ms_out = dp.tile([1, NCORES, 2], F32)
        nc.sync.dma_start(out=ms_in[:], in_=ms16[:])
        nc.gpsimd.collective_compute(
            "AllReduce", ALU.add, replica_groups=gall,
            ins=[ms_in[:].opt()], outs=[ms_out[:].opt()],
        )
        ms_sb = sg.tile([1, NCORES, 2], F32)
        nc.sync.dma_start(out=ms_sb[:], in_=ms_out[:])

        if mode == "ar2":
            nc.sync.dma_start(
                out=io["out_aw"][:, 0:16].rearrange("p (a b) -> p a b", b=2),
                in_=ms_sb[:],
            )
            nc.sync.dma_start(out=io["out_logp"], in_=logits_sb[:])
            return
        gm = sg.tile([1, 1], F32)
        nc.vector.reduce_max(gm[:], ms_sb[:, :, 0:1], axis=AX.XY)
        ngm = sg.tile([1, 1], F32)
        nc.vector.tensor_scalar_mul(ngm[:], gm[:], -1.0)
        corr = sg.tile([1, NCORES, 1], F32)
        nc.scalar.activation(
            out=corr[:], in_=ms_sb[:, :, 0:1], func=AF.Exp, bias=ngm[:], scale=1.0
        )
        prod = sg.tile([1, NCORES, 1], F32)
        ssum = sg.tile([1, 1], F32)
        # (tensor_tensor_reduce mis-executes on HW via this path; keep it simple)
        nc.vector.tensor_mul(prod[:], corr[:], ms_sb[:, :, 1:2])
        nc.vector.reduce_sum(ssum[:], prod[:], axis=AX.XY)
        lse = sg.tile([1, 1], F32)
        nc.scalar.activation(out=lse[:], in_=ssum[:], func=AF.Ln)
        nc.vector.tensor_add(lse[:], lse[:], gm[:])
        if mode == "merge":
            nc.sync.dma_start(out=io["out_aw"][:, 0:1], in_=lse[:])
            nc.sync.dma_start(out=io["out_logp"], in_=logits_sb[:])
            return

        lse_b = sg.tile([128, 1], F32)
        nc.gpsimd.partition_broadcast(lse_b[:], lse[:])

        nc.vector.tensor_scalar_sub(logp_sb[:], logits_sb[:], lse_b[:])
        nc.sync.dma_start(out=io["out_logp"], in_=logp_sb[:])


def build_module(mode="full", reps=1):
    if ("nc", mode, reps) in _CACHE:
        return _CACHE[("nc", mode, reps)]
    nc = bacc.Bacc(
        "TRN2",
        target_bir_lowering=False,
        debug=False,
        enable_asserts=False,
        num_devices=NCORES,
    )

    def din(name, shape):
        return nc.dram_tensor(name, shape, F32, kind="ExternalInput").ap()

    def dout(name, shape):
        return nc.dram_tensor(name, shape, F32, kind="ExternalOutput").ap()

    io = {
        "cat1": din("cat1", [128, 16]),
        "hfull": din("hfull", [128, KCH]),
        "hown": din("hown", [128, 1]),
        "enc": din("enc", [L, H]),
        "attnw": din("attnw", [16, 128, L]),
        "attnb": din("attnb", [1, L]),
        "combw": din("combw", [16, 128, 128]),
        "combb": din("combb", [128, 1]),
        "wih": din("wih", [24, 128, 128]),
        "whh": din("whh", [24, 128, 128]),
        "brz": din("brz", [128, 16]),
        "bin": din("bin", [128, KCH]),
        "bhn": din("bhn", [128, KCH]),
        "outw": din("outw", [VCH, 128, H]),
        "outb": din("outb", [128, VCH]),
        "onehot": din("onehot", [1, NCORES, 1]),
        "out_logp": dout("out_logp", [128, VCH]),
        "out_h": dout("out_h", [128, KCH]),
        "out_aw": dout("out_aw", [1, L]),
    }

    with tile.TileContext(nc) as tc:
        for r in range(reps):
            _body(tc, io, mode, sfx=f"_{r}" if reps > 1 else "")
    nc.compile()
    _CACHE[("nc", mode, reps)] = nc
    return nc


def prep_inputs(
    input_token, hidden, encoder_outputs, emb, attn_W, attn_b,
    comb_W, comb_b, W_ih, W_hh, b_ih, b_hh, out_W, out_b,
):
    """Host-side sharding/layout. Returns per-core input maps."""
    f = np.float32
    tok = int(np.asarray(input_token).reshape(-1)[0])
    emb_row = np.asarray(emb[tok], dtype=f).reshape(E)
    hvec = np.asarray(hidden, dtype=f).reshape(H)
    cat1 = np.concatenate([emb_row, hvec]).reshape(16, 128).T.copy()
    hfull = hvec.reshape(KCH, 128).T.copy()
    enc = np.ascontiguousarray(np.asarray(encoder_outputs, dtype=f))
    attnw = np.ascontiguousarray(
        np.asarray(attn_W, dtype=f).T.reshape(16, 128, L)
    )
    attnb = np.asarray(attn_b, dtype=f).reshape(1, L).copy()
    b_ih = np.asarray(b_ih, dtype=f)
    b_hh = np.asarray(b_hh, dtype=f)
    brz = np.concatenate(
        [
            (b_ih[0:H] + b_hh[0:H]).reshape(KCH, 128).T,
            (b_ih[H : 2 * H] + b_hh[H : 2 * H]).reshape(KCH, 128).T,
        ],
        axis=1,
    ).copy()
    bin_ = b_ih[2 * H : 3 * H].reshape(KCH, 128).T.copy()
    bhn = b_hh[2 * H : 3 * H].reshape(KCH, 128).T.copy()

    comb_W = np.asarray(comb_W, dtype=f)
    comb_b = np.asarray(comb_b, dtype=f)
    W_ih = np.asarray(W_ih, dtype=f)
    W_hh = np.asarray(W_hh, dtype=f)

    out_W = np.asarray(out_W, dtype=f)
    out_b = np.asarray(out_b, dtype=f)
    vpad = NCORES * VC
    outw_pad = np.zeros((vpad, H), dtype=f)
    outw_pad[:V] = out_W
    outb_pad = np.full((vpad,), NEG_BIG, dtype=f)
    outb_pad[:V] = out_b

    maps = []
    for i in range(NCORES):
        sl = slice(i * 128, (i + 1) * 128)
        combw_t = np.ascontiguousarray(
            comb_W[sl, :].T.reshape(16, 128, 128)
        )
        wih_t = np.ascontiguousarray(
            W_ih[:, sl].reshape(24, 128, 128).transpose(0, 2, 1)
        )
        whh_t = np.ascontiguousarray(
            W_hh[:, sl].reshape(24, 128, 128).transpose(0, 2, 1)
        )
        shard = out_W_shard = outw_pad[i * VC : (i + 1) * VC]
        outw_t = np.ascontiguousarray(
            shard.reshape(VCH, 128, KCH, 128).transpose(0, 3, 2, 1)
        ).reshape(VCH, 128, H)
        outb_t = outb_pad[i * VC : (i + 1) * VC].reshape(VCH, 128).T.copy()
        maps.append(
            {
                "cat1": cat1,
                "hfull": hfull,
                "hown": hvec[sl].reshape(128, 1).copy(),
                "enc": enc,
                "attnw": attnw,
                "attnb": attnb,
                "combw": combw_t,
                "combb": comb_b[sl].reshape(128, 1).copy(),
                "wih": wih_t,
                "whh": whh_t,
                "brz": brz,
                "bin": bin_,
                "bhn": bhn,
                "outw": outw_t,
                "outb": outb_t,
                "onehot": np.eye(NCORES, dtype=f)[i].reshape(1, NCORES, 1).copy(),
            }
        )
    return maps


def finish_outputs(results):
    """Gather/unshard per-core results into reference-shaped outputs."""
    logp = np.concatenate(
        [np.asarray(results[i]["out_logp"]).T.reshape(VC) for i in range(NCORES)]
    )[:V].reshape(1, V)
    h_new = np.asarray(results[0]["out_h"]).T.reshape(1, 1, H)
    attn_w = np.asarray(results[0]["out_aw"]).reshape(1, L)
    return logp.astype(np.float32), h_new.astype(np.float32), attn_w.astype(np.float32)


def run_on_hw(in_maps, trace=False, mode="full", **kw):
    nc = build_module(mode)
    from concourse.bass_interp import get_hw_module

    old = nc.m
    nc.m = get_hw_module(nc.m)
    try:
        return bass_utils.run_bass_kernel_spmd(
            nc, in_maps, core_ids=list(range(NCORES)), trace=trace, **kw
        )
    finally:
        nc.m = old


def kernel(**inputs):
    in_maps = prep_inputs(**inputs)
    res = run_on_hw(in_maps)
    return finish_outputs(res.results)
